# revision 38
# baseline (speedup 1.0000x reference)
"""Multi-head attention (B=16, T=1024, D=768, H=12) on 8 TRN2 NeuronCores.

Strategy: pure data parallelism over the batch — each core computes full MHA
for 2 batch elements. No collectives.

Device kernel design (per core, bf16 compute / fp32 accumulate):
  - Host pre-transposes x to xT[b] = x[b].T ([D, T]) and pre-packs all weights
    in SBUF-ready layouts, cast to bf16.
  - Heads are processed in pairs (2 x HS = 128 = partition width).
  - qT/kT ([128, T], head pair stacked on partitions) come from
    matmul(lhsT=W_pair[dchunk, 128], rhs=xT[dchunk, T]) accumulated over D.
  - S^T[s, t] per head via row-tiled (tile_position) K=64 matmuls packing both
    heads of a pair into the 128-row PE array concurrently.
  - exp via ScalarE activation (scale=1/sqrt(HS) folded in, no max subtraction:
    |S|/8 <= ~3 for this data, exp is safe in fp32->bf16).
  - O^T = v_aug^T @ expS^T with v_aug = [v | ones]: row 64 of the PSUM result
    is the softmax denominator l[t] for free.
  - O psum is immediately staged to SBUF (freeing the psum bank pair fast);
    l -> 1/l via one DMA hop to partition 0 + reciprocal_approx_fast, gpsimd
    partition_broadcast, then two DVE muls normalize into o_allT.
  - y = O_all @ Wp + bp with lhsT = o_allT (naturally produced above),
    emitted per t-half so it overlaps the remaining attention pipeline.

Scheduling: filler work (projections for the next pair/batch, y halves) is
dripped into explicit slots of the exp-paced attention pipeline in emission
order, so the static Tile schedule keeps both TensorE and ScalarE busy.
"""

import os
from contextlib import ExitStack

import numpy as np
import ml_dtypes

import concourse.bacc as bacc
import concourse.bass as bass
import concourse.mybir as mybir
import concourse.tile as tile
from concourse.bass_utils import run_bass_kernel_spmd

BF16 = ml_dtypes.bfloat16

# Full problem dims
B, T_FULL, D_FULL, H, HS = 16, 1024, 768, 12, 64
N_CORES = 8
NB = B // N_CORES  # batch elements per core


def build_mha_nc(nb, t, d, npair, trn_type="TRN2"):
    """Build the Bass program for `nb` batch elements, seq len `t`, model dim
    `d`, `npair` head pairs (each pair = 128 partition lanes)."""
    P = 128
    KC = d // P              # contraction chunks over model dim
    SC = t // P              # s (key position) chunks
    NTH = max(1, t // 512)   # output-column groups for S/O matmuls
    TW = t // NTH            # width of each group (<= 512)
    TC = t // P              # t row chunks for v/y
    D2 = d // 2              # y-proj free-dim split (<= 512 fp32 psum)
    dpair = 2 * HS           # 128
    scale = 1.0 / np.sqrt(HS)

    f32 = mybir.dt.float32
    bf16 = mybir.dt.bfloat16
    AF = mybir.ActivationFunctionType

    nc = bacc.Bacc(trn_type, target_bir_lowering=False, debug=False)

    xt_d = nc.dram_tensor("xt", [nb, d, t], bf16, kind="ExternalInput").ap()
    wq_d = nc.dram_tensor("wq", [P, npair, KC, dpair], bf16, kind="ExternalInput").ap()
    wk_d = nc.dram_tensor("wk", [P, npair, KC, dpair], bf16, kind="ExternalInput").ap()
    wv_d = nc.dram_tensor("wv", [P, KC, npair * dpair], bf16, kind="ExternalInput").ap()
    wp_d = nc.dram_tensor("wp", [P, KC, d], bf16, kind="ExternalInput").ap()
    bqk_d = nc.dram_tensor("bqk", [P, npair, 2], f32, kind="ExternalInput").ap()
    bv_d = nc.dram_tensor("bv", [P, npair, dpair], bf16, kind="ExternalInput").ap()
    bp_d = nc.dram_tensor("bp", [P, d], f32, kind="ExternalInput").ap()
    y_d = nc.dram_tensor("y", [nb, t, d], f32, kind="ExternalOutput").ap()

    with TileOrExit(nc) as (tc, ctx):
        # ---- persistent weights (one bufs=1 pool; each tag allocated once) ----
        p_w = ctx.enter_context(tc.tile_pool(name="p_w", bufs=1))
        wq_sb = p_w.tile([P, npair, KC, dpair], bf16, tag="wq", name="wq_sb")
        wk_sb = p_w.tile([P, npair, KC, dpair], bf16, tag="wk", name="wk_sb")
        wv_sb = p_w.tile([P, KC, npair * dpair], bf16, tag="wv", name="wv_sb")
        wp_sb = p_w.tile([P, KC, d], bf16, tag="wp", name="wp_sb")
        bqk_sb = p_w.tile([P, npair, 2], f32, tag="bqk", name="bqk_sb")
        bv_sb = p_w.tile([P, npair, dpair], bf16, tag="bv", name="bv_sb")
        bp_sb = p_w.tile([P, d], f32, tag="bp", name="bp_sb")
        # ---- pools ----
        p_xt = ctx.enter_context(tc.tile_pool(name="p_xt", bufs=2))
        p_vall = ctx.enter_context(tc.tile_pool(name="p_vall", bufs=2))
        p_qk = ctx.enter_context(tc.tile_pool(name="p_qk", bufs=4))
        p_es = ctx.enter_context(tc.tile_pool(name="p_es", bufs=2))
        p_oall = ctx.enter_context(tc.tile_pool(name="p_oall", bufs=2))
        p_norm = ctx.enter_context(tc.tile_pool(name="p_norm", bufs=2))
        p_y = ctx.enter_context(tc.tile_pool(name="p_y", bufs=2))
        ps_s = ctx.enter_context(tc.tile_pool(name="ps_s", bufs=2, space="PSUM"))
        ps_o = ctx.enter_context(tc.tile_pool(name="ps_o", bufs=2, space="PSUM"))
        ps_m = ctx.enter_context(tc.tile_pool(name="ps_m", bufs=2, space="PSUM"))

        # weight loads ride the gpsimd DMA queue so the sync queue is free for
        # xt (first compute dependency). Pair-0 q/k weights go first: they
        # gate the first S matmuls and hence the whole exp pipeline.
        nc.gpsimd.dma_start(wq_sb[:, 0], wq_d[:, 0])
        nc.gpsimd.dma_start(wk_sb[:, 0], wk_d[:, 0])
        nc.gpsimd.dma_start(bqk_sb[:], bqk_d)
        # batch-0 xt rides BOTH dma queues (even chunks on sync, odd on
        # gpsimd ahead of the v/p weights) -- the head is HBM-bound with all
        # 8 cores loading at once, so two engines halve the wait
        xt0 = p_xt.tile([P, KC, t], bf16, tag="xt", name="xt_sb")
        xt0_src = xt_d[0].rearrange("(c p) t -> p c t", p=P)
        for c in range(KC):
            eng = nc.sync if c % 2 == 0 else nc.gpsimd
            eng.dma_start(xt0[:, c], xt0_src[:, c])
        for c in range(KC):
            nc.gpsimd.dma_start(wv_sb[:, c], wv_d[:, c])
        nc.gpsimd.dma_start(bv_sb[:], bv_d)
        for pr in range(1, npair):
            nc.gpsimd.dma_start(wq_sb[:, pr], wq_d[:, pr])
            nc.gpsimd.dma_start(wk_sb[:, pr], wk_d[:, pr])
        nc.gpsimd.dma_start(wp_sb[:], wp_d)
        nc.gpsimd.dma_start(bp_sb[:], bp_d)

        # HAM warm-up: a burst of dummy matmuls during the initial DMA wait
        # so the PE clock is at 2.4 GHz when real work arrives
        warm = p_norm.tile([P, TW], bf16, tag="warm", name="warm")
        nc.vector.memset(warm[:], 0.0)
        wps = ps_m.tile([P, TW], f32, tag="m", name="wps")
        for i in range(20):
            nc.tensor.matmul(
                wps[:], lhsT=warm[:, 0:P], rhs=warm[:], start=(i == 0), stop=(i == 19)
            )

        xts = [None] * nb

        def load_xt(b):
            xt = p_xt.tile([P, KC, t], bf16, tag="xt", name="xt_sb")
            xt_src = xt_d[b].rearrange("(c p) t -> p c t", p=P)
            for c in range(KC):
                nc.sync.dma_start(xt[:, c], xt_src[:, c])
            xts[b] = xt

        def emit_qk(b, pr, qT, kT):
            """Full-t q and k projections for one head pair (24 matmuls)."""
            xt = xts[b]
            for w_sb, bj, dstT in ((wk_sb, 1, kT), (wq_sb, 0, qT)):
                for th in range(NTH):
                    psq = ps_m.tile([P, TW], f32, tag="m", name="psq")
                    for c in range(KC):
                        nc.tensor.matmul(
                            psq[:],
                            lhsT=w_sb[:, pr, c, :],
                            rhs=xt[:, c, th * TW : (th + 1) * TW],
                            start=(c == 0),
                            stop=(c == KC - 1),
                        )
                    nc.vector.tensor_scalar_add(
                        out=dstT[:, th * TW : (th + 1) * TW],
                        in0=psq[:],
                        scalar1=bqk_sb[:, pr, bj : bj + 1],
                    )

        VW = npair * 130 + 64  # trailing pad so every O lhsT can read 128 cols

        def alloc_vall():
            """v_all[:, sc, pair, 0:65] = [v_h0 | ones], [65:130] = [v_h1 | ones];
            the ones row makes row 64 of the O matmul the softmax denominator.
            O lhsT slices are read 128 wide (spilling into the next pair /
            trailing pad) so LDWEIGHTS qualifies for fast weight load; the
            junk output rows 65:127 are never read."""
            v_flat = p_vall.tile([P, SC, VW], bf16, tag="vall", name="v_all")
            vv = v_flat[:, :, 0 : npair * 130].rearrange(
                "p s (r x) -> p s r x", x=130
            )
            ones_view = vv.rearrange("p s r (h x) -> p s r h x", h=2)
            nc.vector.memset(ones_view[:, :, :, :, 64:65], 1.0)
            nc.vector.memset(v_flat[:, :, npair * 130 : VW], 0.0)
            return v_flat, vv

        def emit_v(b, tci, v_pair):
            """v projection for one s-chunk of v_all."""
            v_flat, v_all = v_pair
            xt = xts[b]
            nhalf = (npair + 2) // 3
            for g in range(nhalf):
                gn = min(3, npair - 3 * g)
                psv = ps_m.tile([P, 3 * dpair], f32, tag="m", name="psv")
                for c in range(KC):
                    nc.tensor.matmul(
                        psv[:, : gn * dpair],
                        lhsT=xt[:, c, tci * P : (tci + 1) * P],
                        rhs=wv_sb[:, c, 3 * g * dpair : (3 * g + gn) * dpair],
                        start=(c == 0),
                        stop=(c == KC - 1),
                    )
                glo = 3 * g
                dst = v_all[:, tci, glo : glo + gn, :].rearrange(
                    "p r (h x) -> p r h x", h=2
                )[:, :, :, 0:64]
                src = psv[:, : gn * dpair].rearrange("p (r h e) -> p r h e", r=gn, h=2)
                bias = bv_sb[:, glo : glo + gn, :].rearrange("p r (h e) -> p r h e", h=2)
                nc.vector.tensor_add(out=dst, in0=src, in1=bias)

        def emit_y(b, tci, o_ths):
            """Output projection + bias + store for one t row chunk."""
            thx, tcl = tci // (TC // NTH), tci % (TC // NTH)
            y_sb = p_y.tile([P, d], f32, tag="y", name="y_sb")
            for j in range(2):
                psy = ps_m.tile([P, D2], f32, tag="m", name="psy")
                for c in range(KC):
                    nc.tensor.matmul(
                        psy[:],
                        lhsT=o_ths[thx][:, c, tcl * P : (tcl + 1) * P],
                        rhs=wp_sb[:, c, j * D2 : (j + 1) * D2],
                        start=(c == 0),
                        stop=(c == KC - 1),
                    )
                nc.vector.tensor_add(
                    out=y_sb[:, j * D2 : (j + 1) * D2],
                    in0=psy[:],
                    in1=bp_sb[:, j * D2 : (j + 1) * D2],
                )
            nc.sync.dma_start(out=y_d[b, tci * P : (tci + 1) * P, :], in_=y_sb[:])

        def emit_pipeline(b, pr, th, qT, kT, v_pair, o_th, drip):
            """S -> exp -> O -> normalize for one (pair, th) unit. `drip` is a
            list of filler closures; one is emitted after each s-chunk
            iteration so independent matmul work interleaves with the
            exp-paced pipeline."""
            v_flat, _ = v_pair
            es = p_es.tile([P, SC, 2, TW], bf16, tag="es", name="es")
            psos = [ps_o.tile([P, TW], f32, tag="o", name="pso") for _ in range(2)]
            for sc in range(SC + 2):
                if sc < SC:
                    ps = ps_s.tile([P, 2, TW], f32, tag="s", name="ps_s")
                    nc.tensor.matmul(
                        ps[:, 0, :],
                        lhsT=kT[0:64, sc * P : (sc + 1) * P],
                        rhs=qT[0:64, th * TW : (th + 1) * TW],
                        start=True,
                        stop=True,
                    )
                    nc.tensor.matmul(
                        ps[:, 1, :],
                        lhsT=kT[64:128, sc * P : (sc + 1) * P],
                        rhs=qT[64:128, th * TW : (th + 1) * TW],
                        start=True,
                        stop=True,
                        tile_position=(64, 0),
                    )
                    nc.scalar.activation(
                        out=es[:, sc, :, :], in_=ps[:], func=AF.Exp, scale=scale
                    )
                if sc >= 2:
                    so = sc - 2
                    for h in range(2):
                        off = pr * 130 + 65 * h
                        nc.tensor.matmul(
                            psos[h][:],
                            lhsT=v_flat[:, so, off : off + 128],
                            rhs=es[:, so, h, :],
                            start=(so == 0),
                            stop=(so == SC - 1),
                        )
                if sc < len(drip):
                    drip[sc]()
            # Drain psum to SBUF immediately (frees the O psum bank pair for
            # the next unit within ~1.4us), then compute 1/l and normalize
            # off the staged copy.
            # per-head sub-chains (copy -> l-row DMA -> 1/l -> broadcast)
            # pipeline independently, shortening the critical path to the
            # normalize muls by ~one copy+recip
            stage = p_norm.tile([65, 2, TW], f32, tag="stage", name="stage")
            lg = p_norm.tile([1, 2, TW], f32, tag="lg", name="lg")
            lginv = p_norm.tile([1, 2, TW], f32, tag="lginv", name="lginv")
            linv = p_norm.tile([64, 2, TW], f32, tag="linv", name="linv")
            for h in range(2):
                nc.vector.tensor_copy(out=stage[:, h, :], in_=psos[h][0:65, :])
                nc.sync.dma_start(out=lg[0:1, h, :], in_=stage[64:65, h, :])
                nc.vector.reciprocal_approx_fast(
                    out=lginv[0:1, h, :], in_=lg[0:1, h, :]
                )
                nc.gpsimd.partition_broadcast(
                    out_ap=linv[:, h, :], in_ap=lginv[0:1, h, :], channels=64
                )
            nc.vector.tensor_mul(
                out=o_th[0:64, pr, :], in0=stage[0:64, 0, :], in1=linv[:, 0, :]
            )
            ot = p_norm.tile([64, TW], bf16, tag="ot", name="ot")
            nc.vector.tensor_mul(out=ot[:], in0=stage[0:64, 1, :], in1=linv[:, 1, :])
            nc.sync.dma_start(out=o_th[64:128, pr, :], in_=ot[:])

        # ================= emission =================
        xts[0] = xt0
        qks = {}

        def alloc_qk():
            return (
                p_qk.tile([P, t], bf16, tag="qT", name="qT"),
                p_qk.tile([P, t], bf16, tag="kT", name="kT"),
            )

        qks[(0, 0)] = alloc_qk()
        emit_qk(0, 0, *qks[(0, 0)])
        v_alls = [None] * nb
        v_alls[0] = alloc_vall()
        emit_v(0, 0, v_alls[0])
        emit_v(0, 1, v_alls[0])
        if nb > 1:
            load_xt(1)
        o_ths_all = [None] * nb

        for b in range(nb):
            v_all = v_alls[b]
            o_ths = [
                p_oall.tile([P, npair, TW], bf16, tag=f"oth{th}", name=f"o_th{th}")
                for th in range(NTH)
            ]
            o_ths_all[b] = o_ths
            for pr in range(npair):
                qT, kT = qks[(b, pr)]
                for th in range(NTH):
                    drip = []
                    if pr == 0 and b == 0 and th == 0:
                        drip = [
                            (lambda i=i: emit_v(0, i, v_alls[0])) for i in range(2, SC)
                        ]
                    if pr == 0 and b > 0 and th == 0:
                        # y for previous batch's second half + this batch's v
                        drip = [
                            (lambda i=i: emit_y(b - 1, i, o_ths_all[b - 1]))
                            for i in range(TC // 2, TC)
                        ]
                    if pr == 3 and th == 0 and b + 1 < nb:
                        # next batch's v projection (xt already loaded)
                        if v_alls[b + 1] is None:
                            v_alls[b + 1] = alloc_vall()
                        drip = [
                            (lambda i=i: emit_v(b + 1, i, v_alls[b + 1]))
                            for i in range(SC)
                        ]
                    if pr == 4 and th == 1 and b + 1 < nb:
                        # next batch's first qk pair (tiles allocated lazily
                        # here so they don't pin the qT/kT ring all batch)
                        def _qk_next(bn=b + 1):
                            qks[(bn, 0)] = alloc_qk()
                            emit_qk(bn, 0, *qks[(bn, 0)])

                        drip = [_qk_next]
                    if pr == npair - 1 and th == 1:
                        # first-half y as soon as th0 columns are complete
                        drip = [
                            (lambda i=i: emit_y(b, i, o_ths)) for i in range(TC // 2)
                        ]
                    emit_pipeline(b, pr, th, qT, kT, v_all, o_ths[th], drip)
                    if th == 1 and pr + 1 < npair:
                        qks[(b, pr + 1)] = alloc_qk()
                        emit_qk(b, pr + 1, *qks[(b, pr + 1)])
            if b == nb - 1:
                for tci in range(TC // 2, TC):
                    emit_y(b, tci, o_ths)

    nc.compile()
    return nc


class TileOrExit:
    """Combined TileContext + ExitStack context manager."""

    def __init__(self, nc):
        self.nc = nc
        self.ctx = ExitStack()
        self.tc = tile.TileContext(nc)

    def __enter__(self):
        self.ctx.__enter__()
        self.tc.__enter__()
        return self.tc, self.ctx

    def __exit__(self, *a):
        # close pools before TileContext exits scheduling
        self.ctx.__exit__(*a)
        return self.tc.__exit__(*a)


def prep_inputs(x, Wq, bq, Wk, bk, Wv, bv, Wp, bp, nb, npair):
    """Host-side packing into the DRAM layouts the device kernel expects.

    Returns a list of per-core input maps."""
    P = 128
    t = x.shape[1]
    d = x.shape[2]
    KC = d // P
    dpair = 2 * HS

    def to_bf(a):
        return np.ascontiguousarray(a).astype(BF16)

    # x^T per batch element
    xt = np.ascontiguousarray(x.transpose(0, 2, 1)).astype(BF16)  # [B, d, t]

    # wq/wk: [P, pair, c, 128] with cols 0:64 = head 2p, 64:128 = head 2p+1
    def pack_qk(W):
        # W: [H, d, HS] -> [pair, 2, KC, P, HS] -> [P, pair, KC, 2*HS]
        w = W.reshape(npair, 2, KC, P, HS)
        w = w.transpose(3, 0, 2, 1, 4).reshape(P, npair, KC, dpair)
        return to_bf(w)

    wq = pack_qk(Wq)
    wk = pack_qk(Wk)
    wv = pack_qk(Wv).transpose(0, 2, 1, 3).reshape(P, KC, npair * dpair)
    wv = np.ascontiguousarray(wv)  # [P, c, pair*128]
    # wp: [P, c, d]
    wp = to_bf(Wp.reshape(KC, P, d).transpose(1, 0, 2))
    # bqk: [P, pair, 2] fp32: partition = pair-stacked head dims
    bqk = np.stack(
        [bq.reshape(npair, dpair), bk.reshape(npair, dpair)], axis=-1
    )  # [pair, 128, 2]
    bqk = np.ascontiguousarray(bqk.transpose(1, 0, 2)).astype(np.float32)  # [P, pair, 2]
    # bv broadcast along t partitions: [P, pair, 128]
    bv_bc = np.broadcast_to(bv.reshape(1, npair, dpair), (P, npair, dpair))
    bv_bc = to_bf(bv_bc)
    # bp broadcast: [P, d] fp32
    bp_bc = np.ascontiguousarray(np.broadcast_to(bp.reshape(1, d), (P, d))).astype(
        np.float32
    )

    weights = {
        "wq": wq,
        "wk": wk,
        "wv": wv,
        "wp": wp,
        "bqk": bqk,
        "bv": bv_bc,
        "bp": bp_bc,
    }
    n_cores = x.shape[0] // nb
    in_maps = []
    for i in range(n_cores):
        m = dict(weights)
        m["xt"] = np.ascontiguousarray(xt[i * nb : (i + 1) * nb])
        in_maps.append(m)
    return in_maps


_NC_CACHE = {}
LAST_RESULT = {}


def kernel(x, Wq, bq, Wk, bk, Wv, bv, Wp, bp, _trace=False):
    x = np.asarray(x, dtype=np.float32)
    Wq, bq = np.asarray(Wq, np.float32), np.asarray(bq, np.float32)
    Wk, bk = np.asarray(Wk, np.float32), np.asarray(bk, np.float32)
    Wv, bv = np.asarray(Wv, np.float32), np.asarray(bv, np.float32)
    Wp, bp = np.asarray(Wp, np.float32), np.asarray(bp, np.float32)

    npair = H // 2
    key = ("v3", NB, T_FULL, D_FULL, npair)
    if key not in _NC_CACHE:
        _NC_CACHE[key] = build_mha_nc(NB, T_FULL, D_FULL, npair)
    nc = _NC_CACHE[key]

    in_maps = prep_inputs(x, Wq, bq, Wk, bk, Wv, bv, Wp, bp, NB, npair)
    res = run_bass_kernel_spmd(
        nc, in_maps, core_ids=list(range(N_CORES)), trace=_trace
    )
    LAST_RESULT["exec_time_ns"] = res.exec_time_ns
    LAST_RESULT["res"] = res
    outs = [res.results[i]["y"] for i in range(N_CORES)]
    return np.concatenate(outs, axis=0).astype(np.float32)


# revision 39
# speedup vs baseline: 1.0000x; 1.0000x over previous
"""Multi-head attention (B=16, T=1024, D=768, H=12) on 8 TRN2 NeuronCores.

Strategy: pure data parallelism over the batch — each core computes full MHA
for 2 batch elements. No collectives.

Device kernel design (per core, bf16 compute / fp32 accumulate):
  - Host pre-transposes x to xT[b] = x[b].T ([D, T]) and pre-packs all weights
    in SBUF-ready layouts, cast to bf16.
  - Heads are processed in pairs (2 x HS = 128 = partition width).
  - qT/kT ([128, T], head pair stacked on partitions) come from
    matmul(lhsT=W_pair[dchunk, 128], rhs=xT[dchunk, T]) accumulated over D.
  - S^T[s, t] per head via row-tiled (tile_position) K=64 matmuls packing both
    heads of a pair into the 128-row PE array concurrently.
  - exp via ScalarE activation (scale=1/sqrt(HS) folded in, no max subtraction:
    |S|/8 <= ~3 for this data, exp is safe in fp32->bf16).
  - O^T = v_aug^T @ expS^T with v_aug = [v | ones]: row 64 of the PSUM result
    is the softmax denominator l[t] for free.
  - O psum is immediately staged to SBUF (freeing the psum bank pair fast);
    l -> 1/l via one DMA hop to partition 0 + reciprocal_approx_fast, gpsimd
    partition_broadcast, then two DVE muls normalize into o_allT.
  - y = O_all @ Wp + bp with lhsT = o_allT (naturally produced above),
    emitted per t-half so it overlaps the remaining attention pipeline.

Scheduling: filler work (projections for the next pair/batch, y halves) is
dripped into explicit slots of the exp-paced attention pipeline in emission
order, so the static Tile schedule keeps both TensorE and ScalarE busy.
"""

import os
from contextlib import ExitStack

import numpy as np
import ml_dtypes

import concourse.bacc as bacc
import concourse.bass as bass
import concourse.mybir as mybir
import concourse.tile as tile
from concourse.bass_utils import run_bass_kernel_spmd

BF16 = ml_dtypes.bfloat16

# Full problem dims
B, T_FULL, D_FULL, H, HS = 16, 1024, 768, 12, 64
N_CORES = 8
NB = B // N_CORES  # batch elements per core


def build_mha_nc(nb, t, d, npair, trn_type="TRN2"):
    """Build the Bass program for `nb` batch elements, seq len `t`, model dim
    `d`, `npair` head pairs (each pair = 128 partition lanes)."""
    P = 128
    KC = d // P              # contraction chunks over model dim
    SC = t // P              # s (key position) chunks
    NTH = max(1, t // 512)   # output-column groups for S/O matmuls
    TW = t // NTH            # width of each group (<= 512)
    TC = t // P              # t row chunks for v/y
    D2 = d // 2              # y-proj free-dim split (<= 512 fp32 psum)
    dpair = 2 * HS           # 128
    scale = 1.0 / np.sqrt(HS)

    f32 = mybir.dt.float32
    bf16 = mybir.dt.bfloat16
    AF = mybir.ActivationFunctionType

    nc = bacc.Bacc(trn_type, target_bir_lowering=False, debug=False)

    xt_d = nc.dram_tensor("xt", [nb, d, t], bf16, kind="ExternalInput").ap()
    wq_d = nc.dram_tensor("wq", [P, npair, KC, dpair], bf16, kind="ExternalInput").ap()
    wk_d = nc.dram_tensor("wk", [P, npair, KC, dpair], bf16, kind="ExternalInput").ap()
    wv_d = nc.dram_tensor("wv", [P, KC, npair * dpair], bf16, kind="ExternalInput").ap()
    wp_d = nc.dram_tensor("wp", [P, KC, d], bf16, kind="ExternalInput").ap()
    bqk_d = nc.dram_tensor("bqk", [P, npair, 2], f32, kind="ExternalInput").ap()
    bv_d = nc.dram_tensor("bv", [P, npair, dpair], bf16, kind="ExternalInput").ap()
    bp_d = nc.dram_tensor("bp", [P, d], f32, kind="ExternalInput").ap()
    y_d = nc.dram_tensor("y", [nb, t, d], f32, kind="ExternalOutput").ap()

    with TileOrExit(nc) as (tc, ctx):
        # ---- persistent weights (one bufs=1 pool; each tag allocated once) ----
        p_w = ctx.enter_context(tc.tile_pool(name="p_w", bufs=1))
        wq_sb = p_w.tile([P, npair, KC, dpair], bf16, tag="wq", name="wq_sb")
        wk_sb = p_w.tile([P, npair, KC, dpair], bf16, tag="wk", name="wk_sb")
        wv_sb = p_w.tile([P, KC, npair * dpair], bf16, tag="wv", name="wv_sb")
        wp_sb = p_w.tile([P, KC, d], bf16, tag="wp", name="wp_sb")
        bqk_sb = p_w.tile([P, npair, 2], f32, tag="bqk", name="bqk_sb")
        bv_sb = p_w.tile([P, npair, dpair], bf16, tag="bv", name="bv_sb")
        bp_sb = p_w.tile([P, d], f32, tag="bp", name="bp_sb")
        # weight loads ride the gpsimd DMA queue so the sync queue is free for
        # xt (first compute dependency). Pair-0 q/k weights go first: they
        # gate the first S matmuls and hence the whole exp pipeline.
        nc.gpsimd.dma_start(wq_sb[:, 0], wq_d[:, 0])
        nc.gpsimd.dma_start(wk_sb[:, 0], wk_d[:, 0])
        nc.gpsimd.dma_start(bqk_sb[:], bqk_d)
        for c in range(KC):
            nc.gpsimd.dma_start(wv_sb[:, c], wv_d[:, c])
        nc.gpsimd.dma_start(bv_sb[:], bv_d)
        for pr in range(1, npair):
            nc.gpsimd.dma_start(wq_sb[:, pr], wq_d[:, pr])
            nc.gpsimd.dma_start(wk_sb[:, pr], wk_d[:, pr])
        nc.gpsimd.dma_start(wp_sb[:], wp_d)
        nc.gpsimd.dma_start(bp_sb[:], bp_d)

        # ---- pools ----
        p_xt = ctx.enter_context(tc.tile_pool(name="p_xt", bufs=2))
        p_vall = ctx.enter_context(tc.tile_pool(name="p_vall", bufs=2))
        p_qk = ctx.enter_context(tc.tile_pool(name="p_qk", bufs=4))
        p_es = ctx.enter_context(tc.tile_pool(name="p_es", bufs=2))
        p_oall = ctx.enter_context(tc.tile_pool(name="p_oall", bufs=2))
        p_norm = ctx.enter_context(tc.tile_pool(name="p_norm", bufs=2))
        p_y = ctx.enter_context(tc.tile_pool(name="p_y", bufs=2))
        ps_s = ctx.enter_context(tc.tile_pool(name="ps_s", bufs=2, space="PSUM"))
        ps_o = ctx.enter_context(tc.tile_pool(name="ps_o", bufs=2, space="PSUM"))
        ps_m = ctx.enter_context(tc.tile_pool(name="ps_m", bufs=2, space="PSUM"))

        # HAM warm-up: a burst of dummy matmuls during the initial DMA wait
        # so the PE clock is at 2.4 GHz when real work arrives
        warm = p_norm.tile([P, TW], bf16, tag="warm", name="warm")
        nc.vector.memset(warm[:], 0.0)
        wps = ps_m.tile([P, TW], f32, tag="m", name="wps")
        for i in range(20):
            nc.tensor.matmul(
                wps[:], lhsT=warm[:, 0:P], rhs=warm[:], start=(i == 0), stop=(i == 19)
            )

        xts = [None] * nb

        def load_xt(b):
            xt = p_xt.tile([P, KC, t], bf16, tag="xt", name="xt_sb")
            xt_src = xt_d[b].rearrange("(c p) t -> p c t", p=P)
            for c in range(KC):
                nc.sync.dma_start(xt[:, c], xt_src[:, c])
            xts[b] = xt

        def emit_qk(b, pr, qT, kT):
            """Full-t q and k projections for one head pair (24 matmuls)."""
            xt = xts[b]
            for w_sb, bj, dstT in ((wk_sb, 1, kT), (wq_sb, 0, qT)):
                for th in range(NTH):
                    psq = ps_m.tile([P, TW], f32, tag="m", name="psq")
                    for c in range(KC):
                        nc.tensor.matmul(
                            psq[:],
                            lhsT=w_sb[:, pr, c, :],
                            rhs=xt[:, c, th * TW : (th + 1) * TW],
                            start=(c == 0),
                            stop=(c == KC - 1),
                        )
                    nc.vector.tensor_scalar_add(
                        out=dstT[:, th * TW : (th + 1) * TW],
                        in0=psq[:],
                        scalar1=bqk_sb[:, pr, bj : bj + 1],
                    )

        VW = npair * 130 + 64  # trailing pad so every O lhsT can read 128 cols

        def alloc_vall():
            """v_all[:, sc, pair, 0:65] = [v_h0 | ones], [65:130] = [v_h1 | ones];
            the ones row makes row 64 of the O matmul the softmax denominator.
            O lhsT slices are read 128 wide (spilling into the next pair /
            trailing pad) so LDWEIGHTS qualifies for fast weight load; the
            junk output rows 65:127 are never read."""
            v_flat = p_vall.tile([P, SC, VW], bf16, tag="vall", name="v_all")
            vv = v_flat[:, :, 0 : npair * 130].rearrange(
                "p s (r x) -> p s r x", x=130
            )
            ones_view = vv.rearrange("p s r (h x) -> p s r h x", h=2)
            nc.vector.memset(ones_view[:, :, :, :, 64:65], 1.0)
            nc.vector.memset(v_flat[:, :, npair * 130 : VW], 0.0)
            return v_flat, vv

        def emit_v(b, tci, v_pair):
            """v projection for one s-chunk of v_all."""
            v_flat, v_all = v_pair
            xt = xts[b]
            nhalf = (npair + 2) // 3
            for g in range(nhalf):
                gn = min(3, npair - 3 * g)
                psv = ps_m.tile([P, 3 * dpair], f32, tag="m", name="psv")
                for c in range(KC):
                    nc.tensor.matmul(
                        psv[:, : gn * dpair],
                        lhsT=xt[:, c, tci * P : (tci + 1) * P],
                        rhs=wv_sb[:, c, 3 * g * dpair : (3 * g + gn) * dpair],
                        start=(c == 0),
                        stop=(c == KC - 1),
                    )
                glo = 3 * g
                dst = v_all[:, tci, glo : glo + gn, :].rearrange(
                    "p r (h x) -> p r h x", h=2
                )[:, :, :, 0:64]
                src = psv[:, : gn * dpair].rearrange("p (r h e) -> p r h e", r=gn, h=2)
                bias = bv_sb[:, glo : glo + gn, :].rearrange("p r (h e) -> p r h e", h=2)
                nc.vector.tensor_add(out=dst, in0=src, in1=bias)

        def emit_y(b, tci, o_ths):
            """Output projection + bias + store for one t row chunk."""
            thx, tcl = tci // (TC // NTH), tci % (TC // NTH)
            y_sb = p_y.tile([P, d], f32, tag="y", name="y_sb")
            for j in range(2):
                psy = ps_m.tile([P, D2], f32, tag="m", name="psy")
                for c in range(KC):
                    nc.tensor.matmul(
                        psy[:],
                        lhsT=o_ths[thx][:, c, tcl * P : (tcl + 1) * P],
                        rhs=wp_sb[:, c, j * D2 : (j + 1) * D2],
                        start=(c == 0),
                        stop=(c == KC - 1),
                    )
                nc.vector.tensor_add(
                    out=y_sb[:, j * D2 : (j + 1) * D2],
                    in0=psy[:],
                    in1=bp_sb[:, j * D2 : (j + 1) * D2],
                )
            nc.sync.dma_start(out=y_d[b, tci * P : (tci + 1) * P, :], in_=y_sb[:])

        def emit_pipeline(b, pr, th, qT, kT, v_pair, o_th, drip):
            """S -> exp -> O -> normalize for one (pair, th) unit. `drip` is a
            list of filler closures; one is emitted after each s-chunk
            iteration so independent matmul work interleaves with the
            exp-paced pipeline."""
            v_flat, _ = v_pair
            es = p_es.tile([P, SC, 2, TW], bf16, tag="es", name="es")
            psos = [ps_o.tile([P, TW], f32, tag="o", name="pso") for _ in range(2)]
            for sc in range(SC + 2):
                if sc < SC:
                    ps = ps_s.tile([P, 2, TW], f32, tag="s", name="ps_s")
                    nc.tensor.matmul(
                        ps[:, 0, :],
                        lhsT=kT[0:64, sc * P : (sc + 1) * P],
                        rhs=qT[0:64, th * TW : (th + 1) * TW],
                        start=True,
                        stop=True,
                    )
                    nc.tensor.matmul(
                        ps[:, 1, :],
                        lhsT=kT[64:128, sc * P : (sc + 1) * P],
                        rhs=qT[64:128, th * TW : (th + 1) * TW],
                        start=True,
                        stop=True,
                        tile_position=(64, 0),
                    )
                    nc.scalar.activation(
                        out=es[:, sc, :, :], in_=ps[:], func=AF.Exp, scale=scale
                    )
                if sc >= 2:
                    so = sc - 2
                    for h in range(2):
                        off = pr * 130 + 65 * h
                        nc.tensor.matmul(
                            psos[h][:],
                            lhsT=v_flat[:, so, off : off + 128],
                            rhs=es[:, so, h, :],
                            start=(so == 0),
                            stop=(so == SC - 1),
                        )
                if sc < len(drip):
                    drip[sc]()
            # Drain psum to SBUF immediately (frees the O psum bank pair for
            # the next unit within ~1.4us), then compute 1/l and normalize
            # off the staged copy.
            # per-head sub-chains (copy -> l-row DMA -> 1/l -> broadcast)
            # pipeline independently, shortening the critical path to the
            # normalize muls by ~one copy+recip
            stage = p_norm.tile([65, 2, TW], f32, tag="stage", name="stage")
            lg = p_norm.tile([1, 2, TW], f32, tag="lg", name="lg")
            lginv = p_norm.tile([1, 2, TW], f32, tag="lginv", name="lginv")
            linv = p_norm.tile([64, 2, TW], f32, tag="linv", name="linv")
            for h in range(2):
                nc.vector.tensor_copy(out=stage[:, h, :], in_=psos[h][0:65, :])
                nc.sync.dma_start(out=lg[0:1, h, :], in_=stage[64:65, h, :])
                nc.vector.reciprocal_approx_fast(
                    out=lginv[0:1, h, :], in_=lg[0:1, h, :]
                )
                nc.gpsimd.partition_broadcast(
                    out_ap=linv[:, h, :], in_ap=lginv[0:1, h, :], channels=64
                )
            nc.vector.tensor_mul(
                out=o_th[0:64, pr, :], in0=stage[0:64, 0, :], in1=linv[:, 0, :]
            )
            ot = p_norm.tile([64, TW], bf16, tag="ot", name="ot")
            nc.vector.tensor_mul(out=ot[:], in0=stage[0:64, 1, :], in1=linv[:, 1, :])
            nc.sync.dma_start(out=o_th[64:128, pr, :], in_=ot[:])

        # ================= emission =================
        load_xt(0)
        qks = {}

        def alloc_qk():
            return (
                p_qk.tile([P, t], bf16, tag="qT", name="qT"),
                p_qk.tile([P, t], bf16, tag="kT", name="kT"),
            )

        qks[(0, 0)] = alloc_qk()
        emit_qk(0, 0, *qks[(0, 0)])
        v_alls = [None] * nb
        v_alls[0] = alloc_vall()
        emit_v(0, 0, v_alls[0])
        emit_v(0, 1, v_alls[0])
        if nb > 1:
            load_xt(1)
        o_ths_all = [None] * nb

        for b in range(nb):
            v_all = v_alls[b]
            o_ths = [
                p_oall.tile([P, npair, TW], bf16, tag=f"oth{th}", name=f"o_th{th}")
                for th in range(NTH)
            ]
            o_ths_all[b] = o_ths
            for pr in range(npair):
                qT, kT = qks[(b, pr)]
                for th in range(NTH):
                    drip = []
                    if pr == 0 and b == 0 and th == 0:
                        drip = [
                            (lambda i=i: emit_v(0, i, v_alls[0])) for i in range(2, SC)
                        ]
                    if pr == 0 and b > 0 and th == 0:
                        # y for previous batch's second half + this batch's v
                        drip = [
                            (lambda i=i: emit_y(b - 1, i, o_ths_all[b - 1]))
                            for i in range(TC // 2, TC)
                        ]
                    if pr == 3 and th == 0 and b + 1 < nb:
                        # next batch's v projection (xt already loaded)
                        if v_alls[b + 1] is None:
                            v_alls[b + 1] = alloc_vall()
                        drip = [
                            (lambda i=i: emit_v(b + 1, i, v_alls[b + 1]))
                            for i in range(SC)
                        ]
                    if pr == 4 and th == 1 and b + 1 < nb:
                        # next batch's first qk pair (tiles allocated lazily
                        # here so they don't pin the qT/kT ring all batch)
                        def _qk_next(bn=b + 1):
                            qks[(bn, 0)] = alloc_qk()
                            emit_qk(bn, 0, *qks[(bn, 0)])

                        drip = [_qk_next]
                    if pr == npair - 1 and th == 1:
                        # first-half y as soon as th0 columns are complete
                        drip = [
                            (lambda i=i: emit_y(b, i, o_ths)) for i in range(TC // 2)
                        ]
                    emit_pipeline(b, pr, th, qT, kT, v_all, o_ths[th], drip)
                    if th == 1 and pr + 1 < npair:
                        qks[(b, pr + 1)] = alloc_qk()
                        emit_qk(b, pr + 1, *qks[(b, pr + 1)])
            if b == nb - 1:
                for tci in range(TC // 2, TC):
                    emit_y(b, tci, o_ths)

    nc.compile()
    return nc


class TileOrExit:
    """Combined TileContext + ExitStack context manager."""

    def __init__(self, nc):
        self.nc = nc
        self.ctx = ExitStack()
        self.tc = tile.TileContext(nc)

    def __enter__(self):
        self.ctx.__enter__()
        self.tc.__enter__()
        return self.tc, self.ctx

    def __exit__(self, *a):
        # close pools before TileContext exits scheduling
        self.ctx.__exit__(*a)
        return self.tc.__exit__(*a)


def prep_inputs(x, Wq, bq, Wk, bk, Wv, bv, Wp, bp, nb, npair):
    """Host-side packing into the DRAM layouts the device kernel expects.

    Returns a list of per-core input maps."""
    P = 128
    t = x.shape[1]
    d = x.shape[2]
    KC = d // P
    dpair = 2 * HS

    def to_bf(a):
        return np.ascontiguousarray(a).astype(BF16)

    # x^T per batch element
    xt = np.ascontiguousarray(x.transpose(0, 2, 1)).astype(BF16)  # [B, d, t]

    # wq/wk: [P, pair, c, 128] with cols 0:64 = head 2p, 64:128 = head 2p+1
    def pack_qk(W):
        # W: [H, d, HS] -> [pair, 2, KC, P, HS] -> [P, pair, KC, 2*HS]
        w = W.reshape(npair, 2, KC, P, HS)
        w = w.transpose(3, 0, 2, 1, 4).reshape(P, npair, KC, dpair)
        return to_bf(w)

    wq = pack_qk(Wq)
    wk = pack_qk(Wk)
    wv = pack_qk(Wv).transpose(0, 2, 1, 3).reshape(P, KC, npair * dpair)
    wv = np.ascontiguousarray(wv)  # [P, c, pair*128]
    # wp: [P, c, d]
    wp = to_bf(Wp.reshape(KC, P, d).transpose(1, 0, 2))
    # bqk: [P, pair, 2] fp32: partition = pair-stacked head dims
    bqk = np.stack(
        [bq.reshape(npair, dpair), bk.reshape(npair, dpair)], axis=-1
    )  # [pair, 128, 2]
    bqk = np.ascontiguousarray(bqk.transpose(1, 0, 2)).astype(np.float32)  # [P, pair, 2]
    # bv broadcast along t partitions: [P, pair, 128]
    bv_bc = np.broadcast_to(bv.reshape(1, npair, dpair), (P, npair, dpair))
    bv_bc = to_bf(bv_bc)
    # bp broadcast: [P, d] fp32
    bp_bc = np.ascontiguousarray(np.broadcast_to(bp.reshape(1, d), (P, d))).astype(
        np.float32
    )

    weights = {
        "wq": wq,
        "wk": wk,
        "wv": wv,
        "wp": wp,
        "bqk": bqk,
        "bv": bv_bc,
        "bp": bp_bc,
    }
    n_cores = x.shape[0] // nb
    in_maps = []
    for i in range(n_cores):
        m = dict(weights)
        m["xt"] = np.ascontiguousarray(xt[i * nb : (i + 1) * nb])
        in_maps.append(m)
    return in_maps


_NC_CACHE = {}
LAST_RESULT = {}


def kernel(x, Wq, bq, Wk, bk, Wv, bv, Wp, bp, _trace=False):
    x = np.asarray(x, dtype=np.float32)
    Wq, bq = np.asarray(Wq, np.float32), np.asarray(bq, np.float32)
    Wk, bk = np.asarray(Wk, np.float32), np.asarray(bk, np.float32)
    Wv, bv = np.asarray(Wv, np.float32), np.asarray(bv, np.float32)
    Wp, bp = np.asarray(Wp, np.float32), np.asarray(bp, np.float32)

    npair = H // 2
    key = ("v3", NB, T_FULL, D_FULL, npair)
    if key not in _NC_CACHE:
        _NC_CACHE[key] = build_mha_nc(NB, T_FULL, D_FULL, npair)
    nc = _NC_CACHE[key]

    in_maps = prep_inputs(x, Wq, bq, Wk, bk, Wv, bv, Wp, bp, NB, npair)
    res = run_bass_kernel_spmd(
        nc, in_maps, core_ids=list(range(N_CORES)), trace=_trace
    )
    LAST_RESULT["exec_time_ns"] = res.exec_time_ns
    LAST_RESULT["res"] = res
    outs = [res.results[i]["y"] for i in range(N_CORES)]
    return np.concatenate(outs, axis=0).astype(np.float32)


# revision 40
# speedup vs baseline: 1.0020x; 1.0020x over previous
"""Multi-head attention (B=16, T=1024, D=768, H=12) on 8 TRN2 NeuronCores.

Strategy: pure data parallelism over the batch — each core computes full MHA
for 2 batch elements. No collectives.

Device kernel design (per core, bf16 compute / fp32 accumulate):
  - Host pre-transposes x to xT[b] = x[b].T ([D, T]) and pre-packs all weights
    in SBUF-ready layouts, cast to bf16.
  - Heads are processed in pairs (2 x HS = 128 = partition width).
  - qT/kT ([128, T], head pair stacked on partitions) come from
    matmul(lhsT=W_pair[dchunk, 128], rhs=xT[dchunk, T]) accumulated over D.
  - S^T[s, t] per head via row-tiled (tile_position) K=64 matmuls packing both
    heads of a pair into the 128-row PE array concurrently.
  - exp via ScalarE activation (scale=1/sqrt(HS) folded in, no max subtraction:
    |S|/8 <= ~3 for this data, exp is safe in fp32->bf16).
  - O^T = v_aug^T @ expS^T with v_aug = [v | ones]: row 64 of the PSUM result
    is the softmax denominator l[t] for free.
  - O psum is immediately staged to SBUF (freeing the psum bank pair fast);
    l -> 1/l via one DMA hop to partition 0 + reciprocal_approx_fast, gpsimd
    partition_broadcast, then two DVE muls normalize into o_allT.
  - y = O_all @ Wp + bp with lhsT = o_allT (naturally produced above),
    emitted per t-half so it overlaps the remaining attention pipeline.

Scheduling: filler work (projections for the next pair/batch, y halves) is
dripped into explicit slots of the exp-paced attention pipeline in emission
order, so the static Tile schedule keeps both TensorE and ScalarE busy.
"""

import os
from contextlib import ExitStack

import numpy as np
import ml_dtypes

import concourse.bacc as bacc
import concourse.bass as bass
import concourse.mybir as mybir
import concourse.tile as tile
from concourse.bass_utils import run_bass_kernel_spmd

BF16 = ml_dtypes.bfloat16

# Full problem dims
B, T_FULL, D_FULL, H, HS = 16, 1024, 768, 12, 64
N_CORES = 8
NB = B // N_CORES  # batch elements per core


def build_mha_nc(nb, t, d, npair, trn_type="TRN2"):
    """Build the Bass program for `nb` batch elements, seq len `t`, model dim
    `d`, `npair` head pairs (each pair = 128 partition lanes)."""
    P = 128
    KC = d // P              # contraction chunks over model dim
    SC = t // P              # s (key position) chunks
    NTH = max(1, t // 512)   # output-column groups for S/O matmuls
    TW = t // NTH            # width of each group (<= 512)
    TC = t // P              # t row chunks for v/y
    D2 = d // 2              # y-proj free-dim split (<= 512 fp32 psum)
    dpair = 2 * HS           # 128
    scale = 1.0 / np.sqrt(HS)

    f32 = mybir.dt.float32
    bf16 = mybir.dt.bfloat16
    AF = mybir.ActivationFunctionType

    nc = bacc.Bacc(trn_type, target_bir_lowering=False, debug=False)

    xt_d = nc.dram_tensor("xt", [nb, d, t], bf16, kind="ExternalInput").ap()
    wq_d = nc.dram_tensor("wq", [P, npair, KC, dpair], bf16, kind="ExternalInput").ap()
    wk_d = nc.dram_tensor("wk", [P, npair, KC, dpair], bf16, kind="ExternalInput").ap()
    wv_d = nc.dram_tensor("wv", [P, KC, npair * dpair], bf16, kind="ExternalInput").ap()
    wp_d = nc.dram_tensor("wp", [P, KC, d], bf16, kind="ExternalInput").ap()
    bqk_d = nc.dram_tensor("bqk", [P, npair, 2], f32, kind="ExternalInput").ap()
    bv_d = nc.dram_tensor("bv", [P, npair, dpair], bf16, kind="ExternalInput").ap()
    bp_d = nc.dram_tensor("bp", [P, d], f32, kind="ExternalInput").ap()
    y_d = nc.dram_tensor("y", [nb, t, d], f32, kind="ExternalOutput").ap()

    with TileOrExit(nc) as (tc, ctx):
        # ---- persistent weights (one bufs=1 pool; each tag allocated once) ----
        p_w = ctx.enter_context(tc.tile_pool(name="p_w", bufs=1))
        wq_sb = p_w.tile([P, npair, KC, dpair], bf16, tag="wq", name="wq_sb")
        wk_sb = p_w.tile([P, npair, KC, dpair], bf16, tag="wk", name="wk_sb")
        wv_sb = p_w.tile([P, KC, npair * dpair], bf16, tag="wv", name="wv_sb")
        wp_sb = p_w.tile([P, KC, d], bf16, tag="wp", name="wp_sb")
        bqk_sb = p_w.tile([P, npair, 2], f32, tag="bqk", name="bqk_sb")
        bv_sb = p_w.tile([P, npair, dpair], bf16, tag="bv", name="bv_sb")
        bp_sb = p_w.tile([P, d], f32, tag="bp", name="bp_sb")
        # weight loads ride the gpsimd DMA queue so the sync queue is free for
        # xt (first compute dependency). Pair-0 q/k weights go first: they
        # gate the first S matmuls and hence the whole exp pipeline.
        nc.gpsimd.dma_start(wq_sb[:, 0], wq_d[:, 0])
        nc.gpsimd.dma_start(wk_sb[:, 0], wk_d[:, 0])
        nc.gpsimd.dma_start(bqk_sb[:], bqk_d)
        for c in range(KC):
            nc.gpsimd.dma_start(wv_sb[:, c], wv_d[:, c])
        nc.gpsimd.dma_start(bv_sb[:], bv_d)
        for pr in range(1, npair):
            nc.gpsimd.dma_start(wq_sb[:, pr], wq_d[:, pr])
            nc.gpsimd.dma_start(wk_sb[:, pr], wk_d[:, pr])
        nc.gpsimd.dma_start(wp_sb[:], wp_d)
        nc.gpsimd.dma_start(bp_sb[:], bp_d)

        # ---- pools ----
        p_xt = ctx.enter_context(tc.tile_pool(name="p_xt", bufs=2))
        p_vall = ctx.enter_context(tc.tile_pool(name="p_vall", bufs=2))
        p_qk = ctx.enter_context(tc.tile_pool(name="p_qk", bufs=4))
        p_es = ctx.enter_context(tc.tile_pool(name="p_es", bufs=2))
        p_oall = ctx.enter_context(tc.tile_pool(name="p_oall", bufs=2))
        p_norm = ctx.enter_context(tc.tile_pool(name="p_norm", bufs=2))
        p_y = ctx.enter_context(tc.tile_pool(name="p_y", bufs=2))
        ps_s = ctx.enter_context(tc.tile_pool(name="ps_s", bufs=2, space="PSUM"))
        ps_o = ctx.enter_context(tc.tile_pool(name="ps_o", bufs=2, space="PSUM"))
        ps_m = ctx.enter_context(tc.tile_pool(name="ps_m", bufs=2, space="PSUM"))

        # HAM warm-up: a burst of dummy matmuls during the initial DMA wait
        # so the PE clock is at 2.4 GHz when real work arrives
        warm = p_norm.tile([P, TW], bf16, tag="warm", name="warm")
        nc.vector.memset(warm[:], 0.0)
        wps = ps_m.tile([P, TW], f32, tag="m", name="wps")
        for i in range(20):
            nc.tensor.matmul(
                wps[:], lhsT=warm[:, 0:P], rhs=warm[:], start=(i == 0), stop=(i == 19)
            )

        xts = [None] * nb

        def load_xt(b):
            xt = p_xt.tile([P, KC, t], bf16, tag="xt", name="xt_sb")
            xt_src = xt_d[b].rearrange("(c p) t -> p c t", p=P)
            for c in range(KC):
                nc.sync.dma_start(xt[:, c], xt_src[:, c])
            xts[b] = xt

        def emit_qk(b, pr, qT, kT):
            """Full-t q and k projections for one head pair (24 matmuls)."""
            xt = xts[b]
            for w_sb, bj, dstT in ((wk_sb, 1, kT), (wq_sb, 0, qT)):
                for th in range(NTH):
                    psq = ps_m.tile([P, TW], f32, tag="m", name="psq")
                    for c in range(KC):
                        nc.tensor.matmul(
                            psq[:],
                            lhsT=w_sb[:, pr, c, :],
                            rhs=xt[:, c, th * TW : (th + 1) * TW],
                            start=(c == 0),
                            stop=(c == KC - 1),
                        )
                    nc.vector.tensor_scalar_add(
                        out=dstT[:, th * TW : (th + 1) * TW],
                        in0=psq[:],
                        scalar1=bqk_sb[:, pr, bj : bj + 1],
                    )

        VW = npair * 130 + 64  # trailing pad so every O lhsT can read 128 cols

        def alloc_vall():
            """v_all[:, sc, pair, 0:65] = [v_h0 | ones], [65:130] = [v_h1 | ones];
            the ones row makes row 64 of the O matmul the softmax denominator.
            O lhsT slices are read 128 wide (spilling into the next pair /
            trailing pad) so LDWEIGHTS qualifies for fast weight load; the
            junk output rows 65:127 are never read."""
            v_flat = p_vall.tile([P, SC, VW], bf16, tag="vall", name="v_all")
            vv = v_flat[:, :, 0 : npair * 130].rearrange(
                "p s (r x) -> p s r x", x=130
            )
            ones_view = vv.rearrange("p s r (h x) -> p s r h x", h=2)
            nc.vector.memset(ones_view[:, :, :, :, 64:65], 1.0)
            nc.vector.memset(v_flat[:, :, npair * 130 : VW], 0.0)
            return v_flat, vv

        def emit_v(b, tci, v_pair):
            """v projection for one s-chunk of v_all."""
            v_flat, v_all = v_pair
            xt = xts[b]
            nhalf = (npair + 2) // 3
            for g in range(nhalf):
                gn = min(3, npair - 3 * g)
                psv = ps_m.tile([P, 3 * dpair], f32, tag="m", name="psv")
                for c in range(KC):
                    nc.tensor.matmul(
                        psv[:, : gn * dpair],
                        lhsT=xt[:, c, tci * P : (tci + 1) * P],
                        rhs=wv_sb[:, c, 3 * g * dpair : (3 * g + gn) * dpair],
                        start=(c == 0),
                        stop=(c == KC - 1),
                    )
                glo = 3 * g
                dst = v_all[:, tci, glo : glo + gn, :].rearrange(
                    "p r (h x) -> p r h x", h=2
                )[:, :, :, 0:64]
                src = psv[:, : gn * dpair].rearrange("p (r h e) -> p r h e", r=gn, h=2)
                bias = bv_sb[:, glo : glo + gn, :].rearrange("p r (h e) -> p r h e", h=2)
                nc.vector.tensor_add(out=dst, in0=src, in1=bias)

        def emit_y(b, tci, o_ths, split_dma=False):
            """Output projection + bias + store for one t row chunk. The
            final (tail) chunks store via half-width DMAs alternated across
            both queues so the last stores don't serialize on one queue."""
            thx, tcl = tci // (TC // NTH), tci % (TC // NTH)
            y_sb = p_y.tile([P, d], f32, tag="y", name="y_sb")
            for j in range(2):
                psy = ps_m.tile([P, D2], f32, tag="m", name="psy")
                for c in range(KC):
                    nc.tensor.matmul(
                        psy[:],
                        lhsT=o_ths[thx][:, c, tcl * P : (tcl + 1) * P],
                        rhs=wp_sb[:, c, j * D2 : (j + 1) * D2],
                        start=(c == 0),
                        stop=(c == KC - 1),
                    )
                nc.vector.tensor_add(
                    out=y_sb[:, j * D2 : (j + 1) * D2],
                    in0=psy[:],
                    in1=bp_sb[:, j * D2 : (j + 1) * D2],
                )
                if split_dma:
                    eng = nc.sync if (tci + j) % 2 == 0 else nc.gpsimd
                    eng.dma_start(
                        out=y_d[b, tci * P : (tci + 1) * P, j * D2 : (j + 1) * D2],
                        in_=y_sb[:, j * D2 : (j + 1) * D2],
                    )
            if not split_dma:
                nc.sync.dma_start(out=y_d[b, tci * P : (tci + 1) * P, :], in_=y_sb[:])

        def emit_pipeline(b, pr, th, qT, kT, v_pair, o_th, drip):
            """S -> exp -> O -> normalize for one (pair, th) unit. `drip` is a
            list of filler closures; one is emitted after each s-chunk
            iteration so independent matmul work interleaves with the
            exp-paced pipeline."""
            v_flat, _ = v_pair
            es = p_es.tile([P, SC, 2, TW], bf16, tag="es", name="es")
            psos = [ps_o.tile([P, TW], f32, tag="o", name="pso") for _ in range(2)]
            for sc in range(SC + 2):
                if sc < SC:
                    ps = ps_s.tile([P, 2, TW], f32, tag="s", name="ps_s")
                    nc.tensor.matmul(
                        ps[:, 0, :],
                        lhsT=kT[0:64, sc * P : (sc + 1) * P],
                        rhs=qT[0:64, th * TW : (th + 1) * TW],
                        start=True,
                        stop=True,
                    )
                    nc.tensor.matmul(
                        ps[:, 1, :],
                        lhsT=kT[64:128, sc * P : (sc + 1) * P],
                        rhs=qT[64:128, th * TW : (th + 1) * TW],
                        start=True,
                        stop=True,
                        tile_position=(64, 0),
                    )
                    nc.scalar.activation(
                        out=es[:, sc, :, :], in_=ps[:], func=AF.Exp, scale=scale
                    )
                if sc >= 2:
                    so = sc - 2
                    for h in range(2):
                        off = pr * 130 + 65 * h
                        nc.tensor.matmul(
                            psos[h][:],
                            lhsT=v_flat[:, so, off : off + 128],
                            rhs=es[:, so, h, :],
                            start=(so == 0),
                            stop=(so == SC - 1),
                        )
                if sc < len(drip):
                    drip[sc]()
            # Drain psum to SBUF immediately (frees the O psum bank pair for
            # the next unit within ~1.4us), then compute 1/l and normalize
            # off the staged copy.
            # per-head sub-chains (copy -> l-row DMA -> 1/l -> broadcast)
            # pipeline independently, shortening the critical path to the
            # normalize muls by ~one copy+recip
            stage = p_norm.tile([65, 2, TW], f32, tag="stage", name="stage")
            lg = p_norm.tile([1, 2, TW], f32, tag="lg", name="lg")
            lginv = p_norm.tile([1, 2, TW], f32, tag="lginv", name="lginv")
            linv = p_norm.tile([64, 2, TW], f32, tag="linv", name="linv")
            for h in range(2):
                nc.vector.tensor_copy(out=stage[:, h, :], in_=psos[h][0:65, :])
                nc.sync.dma_start(out=lg[0:1, h, :], in_=stage[64:65, h, :])
                nc.vector.reciprocal_approx_fast(
                    out=lginv[0:1, h, :], in_=lg[0:1, h, :]
                )
                nc.gpsimd.partition_broadcast(
                    out_ap=linv[:, h, :], in_ap=lginv[0:1, h, :], channels=64
                )
            nc.vector.tensor_mul(
                out=o_th[0:64, pr, :], in0=stage[0:64, 0, :], in1=linv[:, 0, :]
            )
            ot = p_norm.tile([64, TW], bf16, tag="ot", name="ot")
            nc.vector.tensor_mul(out=ot[:], in0=stage[0:64, 1, :], in1=linv[:, 1, :])
            nc.sync.dma_start(out=o_th[64:128, pr, :], in_=ot[:])

        # ================= emission =================
        load_xt(0)
        qks = {}

        def alloc_qk():
            return (
                p_qk.tile([P, t], bf16, tag="qT", name="qT"),
                p_qk.tile([P, t], bf16, tag="kT", name="kT"),
            )

        qks[(0, 0)] = alloc_qk()
        emit_qk(0, 0, *qks[(0, 0)])
        v_alls = [None] * nb
        v_alls[0] = alloc_vall()
        emit_v(0, 0, v_alls[0])
        emit_v(0, 1, v_alls[0])
        if nb > 1:
            load_xt(1)
        o_ths_all = [None] * nb

        for b in range(nb):
            v_all = v_alls[b]
            o_ths = [
                p_oall.tile([P, npair, TW], bf16, tag=f"oth{th}", name=f"o_th{th}")
                for th in range(NTH)
            ]
            o_ths_all[b] = o_ths
            for pr in range(npair):
                qT, kT = qks[(b, pr)]
                for th in range(NTH):
                    drip = []
                    if pr == 0 and b == 0 and th == 0:
                        drip = [
                            (lambda i=i: emit_v(0, i, v_alls[0])) for i in range(2, SC)
                        ]
                    if pr == 0 and b > 0 and th == 0:
                        # y for previous batch's second half + this batch's v
                        drip = [
                            (lambda i=i: emit_y(b - 1, i, o_ths_all[b - 1]))
                            for i in range(TC // 2, TC)
                        ]
                    if pr == 3 and th == 0 and b + 1 < nb:
                        # next batch's v projection (xt already loaded)
                        if v_alls[b + 1] is None:
                            v_alls[b + 1] = alloc_vall()
                        drip = [
                            (lambda i=i: emit_v(b + 1, i, v_alls[b + 1]))
                            for i in range(SC)
                        ]
                    if pr == 4 and th == 1 and b + 1 < nb:
                        # next batch's first qk pair (tiles allocated lazily
                        # here so they don't pin the qT/kT ring all batch)
                        def _qk_next(bn=b + 1):
                            qks[(bn, 0)] = alloc_qk()
                            emit_qk(bn, 0, *qks[(bn, 0)])

                        drip = [_qk_next]
                    if pr == npair - 1 and th == 1:
                        # first-half y as soon as th0 columns are complete
                        drip = [
                            (lambda i=i: emit_y(b, i, o_ths)) for i in range(TC // 2)
                        ]
                    emit_pipeline(b, pr, th, qT, kT, v_all, o_ths[th], drip)
                    if th == 1 and pr + 1 < npair:
                        qks[(b, pr + 1)] = alloc_qk()
                        emit_qk(b, pr + 1, *qks[(b, pr + 1)])
            if b == nb - 1:
                for tci in range(TC // 2, TC):
                    emit_y(b, tci, o_ths, split_dma=True)

    nc.compile()
    return nc


class TileOrExit:
    """Combined TileContext + ExitStack context manager."""

    def __init__(self, nc):
        self.nc = nc
        self.ctx = ExitStack()
        self.tc = tile.TileContext(nc)

    def __enter__(self):
        self.ctx.__enter__()
        self.tc.__enter__()
        return self.tc, self.ctx

    def __exit__(self, *a):
        # close pools before TileContext exits scheduling
        self.ctx.__exit__(*a)
        return self.tc.__exit__(*a)


def prep_inputs(x, Wq, bq, Wk, bk, Wv, bv, Wp, bp, nb, npair):
    """Host-side packing into the DRAM layouts the device kernel expects.

    Returns a list of per-core input maps."""
    P = 128
    t = x.shape[1]
    d = x.shape[2]
    KC = d // P
    dpair = 2 * HS

    def to_bf(a):
        return np.ascontiguousarray(a).astype(BF16)

    # x^T per batch element
    xt = np.ascontiguousarray(x.transpose(0, 2, 1)).astype(BF16)  # [B, d, t]

    # wq/wk: [P, pair, c, 128] with cols 0:64 = head 2p, 64:128 = head 2p+1
    def pack_qk(W):
        # W: [H, d, HS] -> [pair, 2, KC, P, HS] -> [P, pair, KC, 2*HS]
        w = W.reshape(npair, 2, KC, P, HS)
        w = w.transpose(3, 0, 2, 1, 4).reshape(P, npair, KC, dpair)
        return to_bf(w)

    wq = pack_qk(Wq)
    wk = pack_qk(Wk)
    wv = pack_qk(Wv).transpose(0, 2, 1, 3).reshape(P, KC, npair * dpair)
    wv = np.ascontiguousarray(wv)  # [P, c, pair*128]
    # wp: [P, c, d]
    wp = to_bf(Wp.reshape(KC, P, d).transpose(1, 0, 2))
    # bqk: [P, pair, 2] fp32: partition = pair-stacked head dims
    bqk = np.stack(
        [bq.reshape(npair, dpair), bk.reshape(npair, dpair)], axis=-1
    )  # [pair, 128, 2]
    bqk = np.ascontiguousarray(bqk.transpose(1, 0, 2)).astype(np.float32)  # [P, pair, 2]
    # bv broadcast along t partitions: [P, pair, 128]
    bv_bc = np.broadcast_to(bv.reshape(1, npair, dpair), (P, npair, dpair))
    bv_bc = to_bf(bv_bc)
    # bp broadcast: [P, d] fp32
    bp_bc = np.ascontiguousarray(np.broadcast_to(bp.reshape(1, d), (P, d))).astype(
        np.float32
    )

    weights = {
        "wq": wq,
        "wk": wk,
        "wv": wv,
        "wp": wp,
        "bqk": bqk,
        "bv": bv_bc,
        "bp": bp_bc,
    }
    n_cores = x.shape[0] // nb
    in_maps = []
    for i in range(n_cores):
        m = dict(weights)
        m["xt"] = np.ascontiguousarray(xt[i * nb : (i + 1) * nb])
        in_maps.append(m)
    return in_maps


_NC_CACHE = {}
LAST_RESULT = {}


def kernel(x, Wq, bq, Wk, bk, Wv, bv, Wp, bp, _trace=False):
    x = np.asarray(x, dtype=np.float32)
    Wq, bq = np.asarray(Wq, np.float32), np.asarray(bq, np.float32)
    Wk, bk = np.asarray(Wk, np.float32), np.asarray(bk, np.float32)
    Wv, bv = np.asarray(Wv, np.float32), np.asarray(bv, np.float32)
    Wp, bp = np.asarray(Wp, np.float32), np.asarray(bp, np.float32)

    npair = H // 2
    key = ("v3", NB, T_FULL, D_FULL, npair)
    if key not in _NC_CACHE:
        _NC_CACHE[key] = build_mha_nc(NB, T_FULL, D_FULL, npair)
    nc = _NC_CACHE[key]

    in_maps = prep_inputs(x, Wq, bq, Wk, bk, Wv, bv, Wp, bp, NB, npair)
    res = run_bass_kernel_spmd(
        nc, in_maps, core_ids=list(range(N_CORES)), trace=_trace
    )
    LAST_RESULT["exec_time_ns"] = res.exec_time_ns
    LAST_RESULT["res"] = res
    outs = [res.results[i]["y"] for i in range(N_CORES)]
    return np.concatenate(outs, axis=0).astype(np.float32)
